# revision 47
# baseline (speedup 1.0000x reference)
"""Trainium2 Bass kernel for AttentionConv3D — v3.

Per-core pipeline (batch-parallel over 8 cores, bf16/fp8 compute):
  1. qkv = w_qkv @ x on PE; ScalarE copies PSUM into pitched SBUF strips:
     q,k blocks as fp8e4 (x8 scale), v blocks as bf16.
  2. depthwise 3x3 conv:
     - q,k blocks: 9 taps as 5 fp8 DoubleRow paired diag-matmuls on PE
       (two shifted windows per instruction; weights x16 in fp8, center
       tap as an exact hi/lo split), accumulated in PSUM, ScalarE
       copyback. The x128 combined scale cancels in the l2 norm.
     - v blocks: 9 taps SBUF-side: DVE tensor_scalar multiplies, adds
       split between DVE tensor_tensor and GpSimd tensor_tensor,
       accumulating straight into v_sb.
  3. channel attention: PE transposes + one PSUM-resident q@k^T;
     l2-norm/temperature folded into the per-head 32x32 softmax.
  4. proj folded into attn: out = (P @ blockdiag(attn))^T @ v.
"""
import sys

sys.path.insert(0, "/opt/trn_rl_repo")

import numpy as np
import ml_dtypes

import concourse.bass as bass
from concourse import bacc, mybir
from concourse.ap import AP
from concourse.tile import TileContext
from concourse.masks import make_identity

FP32 = mybir.dt.float32
BF16 = mybir.dt.bfloat16
FP8 = mybir.dt.float8e4
AX = mybir.AxisListType
ALU = mybir.AluOpType
ACTF = mybir.ActivationFunctionType
PERF = mybir.MatmulPerfMode

C = 256
H = W = 128
HEADS = 8
CH = C // HEADS  # 32
QC = 3 * C       # 768
QB = QC // 128   # 6 qkv channel blocks
VB = 2           # v blocks
KB = 4           # q,k blocks
S = 8            # image rows per strip
NSTRIP = H // S  # 16
PITCH = W + 2    # 130
N_CORES = 8
EPS = 1e-12

DATA_SCALE = 8.0   # q,k pitched-buffer fp8 scale
W_SCALE = 16.0     # q,k tap-weight fp8 scale

# tap pairs for the fp8 DoubleRow conv (q,k blocks). Last pair is the
# center tap's exact hi/lo split (stride-0 second window).
TAP_PAIRS = ((0, 2), (6, 8), (3, 5), (1, 7))

V_TAPS = (0, 1, 2, 3, 4, 5, 6, 7, 8)

# ---- engine-assignment knobs ----
W8_SCALE = 16.0     # fp8 qkv weight scale (q,k blocks)
V_TAPS_PE = (0, 2, 6)      # v-conv taps on PE (bf16 diag matmuls, PSUM)
# qkv copyback engine per output block (4 q,k + 2 v): 'act' or 'dve'
QKV_CB = ("act", "act", "act", "act", "act", "dve")
CONV_CB = ("act", "act", "act", "act")  # q,k conv copyback per block
SUMSQ_ENG = ("act", "act", "act", "act")  # (q0, q1, k0, k1): 'act'/'dve'


def build_kernel():
    nc = bacc.Bacc("TRN2", target_bir_lowering=False, debug=False,
                   num_devices=N_CORES)

    x_d = nc.dram_tensor("x", [C, H, W], BF16, kind="ExternalInput").ap()
    x8_d = nc.dram_tensor("x8", [2, 128, H, W], FP8, kind="ExternalInput").ap()
    wq8_d = nc.dram_tensor("wq8", [128, 2, KB * 128], FP8,
                           kind="ExternalInput").ap()
    wqT_d = nc.dram_tensor("w_qkvT", [C, QC], BF16, kind="ExternalInput").ap()
    w9_d = nc.dram_tensor("w9", [128, QB * 9], FP32, kind="ExternalInput").ap()
    w4lo_d = nc.dram_tensor("w4lo", [128, KB], FP32, kind="ExternalInput").ap()
    wpT_d = nc.dram_tensor("w_projT", [C, C], BF16, kind="ExternalInput").ap()
    temp_d = nc.dram_tensor("temp_pc", [128, 2], FP32, kind="ExternalInput").ap()
    sel_d = nc.dram_tensor("sel2", [2, 2, 128], FP32, kind="ExternalInput").ap()
    out_d = nc.dram_tensor("out", [C, H, W], BF16, kind="ExternalOutput").ap()

    with TileContext(nc) as tc:
        _body(nc, tc, x_d, x8_d, wq8_d, wqT_d, w9_d, w4lo_d, wpT_d, temp_d,
              out_d, sel_d)
    nc.compile()
    return nc


def _body(nc, tc, x_d, x8_d, wq8_d, wqT_d, w9_d, w4lo_d, wpT_d, temp_d,
          out_d, sel_d):
    from contextlib import ExitStack

    ctx = ExitStack()
    with ctx:
        persist = ctx.enter_context(tc.tile_pool(name="persist", bufs=1))

        # ---- persistent tiles ----
        # strip-0 x + wq8 first on the sync queue: they gate the first
        # matmuls, everything else can trickle in behind them.
        x8_s0 = persist.tile([128, 2, S + 1, W], FP8, tag="x8s0")
        nc.sync.dma_start(out=x8_s0[:],
                          in_=x8_d[:].rearrange("b p h w -> p b h w")
                                   [:, :, 0:S + 1, :])
        wq8_sb = persist.tile([128, 2, KB * 128], FP8, tag="wq8")
        nc.sync.dma_start(out=wq8_sb[:], in_=wq8_d[:])
        x_s0 = persist.tile([128, 2, S + 1, W], BF16, tag="xs0")
        nc.sync.dma_start(out=x_s0[:],
                          in_=x_d[:].rearrange("(b p) h w -> p b h w", b=2)
                                  [:, :, 0:S + 1, :])
        wq_sb = persist.tile([128, 2, QC], BF16, tag="wq")
        nc.sync.dma_start(out=wq_sb[:, 0, :], in_=wqT_d[0:128, :])
        nc.sync.dma_start(out=wq_sb[:, 1, :], in_=wqT_d[128:256, :])
        w9_sb = persist.tile([128, QB * 9], FP32, tag="w9")
        nc.sync.dma_start(out=w9_sb[:], in_=w9_d[:])
        wp_sb = persist.tile([128, 2, C], BF16, tag="wp")
        nc.sync.dma_start(out=wp_sb[:, 0, :], in_=wpT_d[0:128, :])
        nc.sync.dma_start(out=wp_sb[:, 1, :], in_=wpT_d[128:256, :])
        w4lo_sb = persist.tile([128, KB], FP32, tag="w4lo")
        nc.sync.dma_start(out=w4lo_sb[:], in_=w4lo_d[:])
        temp_sb = persist.tile([128, 2], FP32, tag="temp")
        nc.sync.dma_start(out=temp_sb[:], in_=temp_d[:])

        ident_bf = persist.tile([128, 128], BF16, tag="idb")
        make_identity(nc, ident_bf)
        ident_f8 = persist.tile([128, 128], FP8, tag="idf8")
        nc.vector.tensor_copy(out=ident_f8[:], in_=ident_bf[:])
        ident_f32 = persist.tile([128, 128], FP32, tag="idf")
        make_identity(nc, ident_f32)
        sel_row = persist.tile([2, 2, 128], FP32, tag="selr")
        nc.sync.dma_start(out=sel_row[:], in_=sel_d[:])
        scr1 = persist.tile([128, 1], FP32, tag="scr1")
        nc.vector.memset(scr1[:], 1.0)
        nc.scalar.activation(out=scr1[:], in_=scr1[:], func=ACTF.Sqrt)
        nc.scalar.activation(out=scr1[:], in_=scr1[:], func=ACTF.Exp)

        # scaled q,k tap weights (x16) for fp8 diag construction
        w9s = persist.tile([128, KB * 9], FP32, tag="w9s")
        nc.vector.tensor_scalar(out=w9s[:], in0=w9_sb[:, 0:KB * 9],
                                scalar1=W_SCALE, scalar2=None, op0=ALU.mult)

        # diag conv-weight tiles, built lazily (after strip 0's qkv is
        # emitted) so the Vector queue doesn't delay the pipeline start.
        NPAIR = len(TAP_PAIRS) + 1
        pdiag = persist.tile([128, KB, NPAIR, 2, 128], FP8, tag="pdiag")
        vdiag = persist.tile([128, VB, 9, 128], BF16, tag="vdiag")

        def build_diag_tiles():
            for qb in range(KB):
                for pi, (ta, tb) in enumerate(TAP_PAIRS):
                    for j, t in enumerate((ta, tb)):
                        nc.vector.tensor_scalar(
                            out=pdiag[:, qb, pi, j, :], in0=ident_f8[:],
                            scalar1=w9s[:, qb * 9 + t:qb * 9 + t + 1],
                            scalar2=None, op0=ALU.mult)
                # center tap hi/lo pair
                nc.vector.tensor_scalar(
                    out=pdiag[:, qb, NPAIR - 1, 0, :], in0=ident_f8[:],
                    scalar1=w9s[:, qb * 9 + 4:qb * 9 + 5],
                    scalar2=None, op0=ALU.mult)
                nc.vector.tensor_scalar(
                    out=pdiag[:, qb, NPAIR - 1, 1, :], in0=ident_f8[:],
                    scalar1=w4lo_sb[:, qb:qb + 1],
                    scalar2=None, op0=ALU.mult)
            for qb in range(VB):
                for t in range(9):
                    nc.vector.tensor_scalar(
                        out=vdiag[:, qb, t, :], in0=ident_bf[:],
                        scalar1=w9_sb[:, (KB + qb) * 9 + t:
                                      (KB + qb) * 9 + t + 1],
                        scalar2=None, op0=ALU.mult)

        # v storage (full image, bf16)
        v_sb = persist.tile([128, VB, H, W], BF16, tag="vsb")
        stats = persist.tile([128, 4, NSTRIP], FP32, tag="stats")
        bd_pre = persist.tile([128, 2, 128], BF16, tag="bdpre")
        nc.vector.memset(bd_pre[:], 0.0)
        pat = persist.tile([128, 2, 256], BF16, tag="pat")
        smalls = persist.tile([128, 64], FP32, tag="smalls")

        # pitched conv-input buffers: q,k in fp8 (x8), v in bf16
        qk_bufs, v_bufs = [], []
        for i in range(3):
            qk_t = persist.tile([128, KB, S + 2, PITCH], FP8, tag=f"qkb{i}",
                                name=f"qkbuf{i}")
            nc.vector.memset(qk_t[:, :, :, 0:1], 0.0)
            nc.vector.memset(qk_t[:, :, :, PITCH - 1:PITCH], 0.0)
            qk_bufs.append(qk_t)
            v_t = persist.tile([128, VB, S + 2, PITCH], BF16, tag=f"vb{i}",
                               name=f"vbuf{i}")
            nc.vector.memset(v_t[:, :, :, 0:1], 0.0)
            nc.vector.memset(v_t[:, :, :, PITCH - 1:PITCH], 0.0)
            v_bufs.append(v_t)
        sq_scr = persist.tile([128, 2, S * W], BF16, tag="sqscr")

        # ---- phase 1 ----
        p1 = ExitStack()
        with p1:
            xpool = p1.enter_context(tc.tile_pool(name="xpool", bufs=2))
            qkpool = p1.enter_context(tc.tile_pool(name="qkpool", bufs=2))
            ypool = p1.enter_context(tc.tile_pool(name="ypool", bufs=2))
            tppool = p1.enter_context(tc.tile_pool(name="tppool", bufs=4))
            ps_mm = p1.enter_context(tc.tile_pool(name="ps_mm", bufs=3, space="PSUM"))
            ps_cv = p1.enter_context(tc.tile_pool(name="ps_cv", bufs=2, space="PSUM"))
            ps_at = p1.enter_context(tc.tile_pool(name="ps_at", bufs=1, space="PSUM"))

            at_tile = ps_at.tile([128, 512], FP32, tag="attn", name="attn")
            attn_ps = [at_tile[:, 0:128], at_tile[:, 128:256]]

            def emit_qkv(s):
                r0 = s * S
                c_lo = r0 if s == 0 else r0 + 1
                c_hi = min(r0 + S, H - 1)

                if s == 0:
                    x_sb, x8_sb = x_s0, x8_s0
                else:
                    x_sb = xpool.tile([128, 2, S + 1, W], BF16, tag="xs",
                                      name=f"xs{s}")
                    nc.sync.dma_start(
                        out=x_sb[:, :, 0:c_hi - c_lo + 1, :],
                        in_=x_d[:].rearrange("(b p) h w -> p b h w", b=2)
                                [:, :, c_lo:c_hi + 1, :])
                    x8_sb = xpool.tile([128, 2, S + 1, W], FP8, tag="x8s",
                                       name=f"x8s{s}")
                    nc.sync.dma_start(
                        out=x8_sb[:, :, 0:c_hi - c_lo + 1, :],
                        in_=x8_d[:].rearrange("b p h w -> p b h w")
                                 [:, :, c_lo:c_hi + 1, :])

                qk_sb = qk_bufs[s % 3]
                v_b = v_bufs[s % 3]
                if s == 0:
                    nc.vector.memset(qk_sb[:, :, 0, :], 0.0)
                    nc.vector.memset(v_b[:, :, 0, :], 0.0)
                if s == NSTRIP - 1:
                    nc.vector.memset(qk_sb[:, :, S + 1, :], 0.0)
                    nc.vector.memset(v_b[:, :, S + 1, :], 0.0)
                if s > 0:
                    prev_qk = qk_bufs[(s - 1) % 3]
                    prev_v = v_bufs[(s - 1) % 3]
                    nc.vector.tensor_copy(
                        out=qk_sb[:, :, 0:2, :], in_=prev_qk[:, :, S:S + 2, :])
                    nc.vector.tensor_copy(
                        out=v_b[:, :, 0:2, :], in_=prev_v[:, :, S:S + 2, :])

                row = c_lo
                while row <= c_hi:
                    cr = min(4, c_hi - row + 1)
                    npx = cr * W
                    boff = row - (r0 - 1)
                    xoff = row - c_lo
                    for qb in range(QB):
                        mm_ps = ps_mm.tile([128, 512], FP32, tag="mmps",
                                           name=f"mmps{s}_{row}_{qb}")
                        if qb < KB:
                            # fp8 DoubleRow: both 128-channel contraction
                            # blocks in one pass (weights x16 in fp8)
                            nc.tensor.matmul(
                                mm_ps[:, 0:npx],
                                lhsT=wq8_sb[:, :, qb * 128:(qb + 1) * 128],
                                rhs=x8_sb[:, :, xoff:xoff + cr, :],
                                start=True, stop=True,
                                perf_mode=PERF.DoubleRow)
                        else:
                            for kb in range(2):
                                nc.tensor.matmul(
                                    mm_ps[:, 0:npx],
                                    lhsT=wq_sb[:, kb, qb * 128:(qb + 1) * 128],
                                    rhs=x_sb[:, kb, xoff:xoff + cr, :],
                                    start=(kb == 0), stop=(kb == 1))
                        cb_in = mm_ps[:, 0:npx].rearrange("p (r w) -> p r w", w=W)
                        if qb < KB:
                            cb_out = qk_sb[:, qb, boff:boff + cr, 1:1 + W]
                            if QKV_CB[qb] == "act":
                                nc.scalar.activation(out=cb_out, in_=cb_in,
                                                     func=ACTF.Copy,
                                                     scale=DATA_SCALE / W8_SCALE)
                            else:
                                nc.vector.tensor_scalar(
                                    out=cb_out, in0=cb_in,
                                    scalar1=DATA_SCALE / W8_SCALE,
                                    scalar2=None, op0=ALU.mult)
                        else:
                            cb_out = v_b[:, qb - KB, boff:boff + cr, 1:1 + W]
                            if QKV_CB[qb] == "dve":
                                nc.vector.tensor_copy(out=cb_out, in_=cb_in)
                            else:
                                nc.scalar.copy(out=cb_out, in_=cb_in)
                    row += cr

            def emit_rest(s):
                r0 = s * S
                qk_sb = qk_bufs[s % 3]
                v_b = v_bufs[s % 3]
                # q,k conv outputs in one tile [g, qk, j, w] so a single
                # DMA transpose per group covers both tensors.
                qk_st = qkpool.tile([128, 2, 2, S, W], BF16, tag="qkst",
                                    name=f"qkst{s}")
                q_st = qk_st[:, :, 0]
                k_st = qk_st[:, :, 1]
                # --- q,k conv: 5 fp8 DoubleRow pair-passes per block ---
                qk_full = qk_sb[:]
                pstride = qk_full.ap[0][0]
                for qb in range(KB):
                    cv_ps = ps_cv.tile([128, 1024], FP32, tag="cvps",
                                       name=f"cvps{s}_{qb}")
                    pairs = list(TAP_PAIRS) + [(4, 4)]
                    for pi, (ta, tb) in enumerate(pairs):
                        dha, dwa = ta // 3 - 1, ta % 3 - 1
                        dhb, dwb = tb // 3 - 1, tb % 3 - 1
                        delta = (dhb - dha) * PITCH + (dwb - dwa)
                        for cnk in range(2):
                            b0 = 4 * cnk + 1 + dha
                            ref = qk_sb[:, qb, b0:b0 + 4, 1 + dwa:1 + dwa + W]
                            rhs = AP(ref.tensor, ref.offset,
                                     [[pstride, 128], [delta, 2],
                                      [PITCH, 4], [1, 128]])
                            nc.tensor.matmul(
                                cv_ps[:, cnk * 512:(cnk + 1) * 512],
                                lhsT=pdiag[:, qb, pi, :, :],
                                rhs=rhs,
                                start=(pi == 0), stop=(pi == len(pairs) - 1),
                                perf_mode=PERF.DoubleRow)
                    dst = q_st[:, qb, :, :] if qb < 2 else k_st[:, qb - 2, :, :]
                    if CONV_CB[qb] == "act":
                        nc.scalar.copy(
                            out=dst[:],
                            in_=cv_ps[:].rearrange("p (r w) -> p r w", w=W))
                    else:
                        nc.vector.tensor_copy(
                            out=dst[:],
                            in_=cv_ps[:].rearrange("p (r w) -> p r w", w=W))

                # --- v conv: PE-side taps as bf16 diag matmuls accumulated
                # in PSUM; remaining taps on DVE in SBUF; merged per block.
                taps_dve = [t for t in V_TAPS if t not in V_TAPS_PE]
                v_ps = []
                for qb in range(VB):
                    vp = ps_cv.tile([128, 1024], FP32, tag="cvps",
                                    name=f"vps{s}_{qb}")
                    v_ps.append(vp)
                    for pi, t in enumerate(V_TAPS_PE):
                        dh, dw = t // 3 - 1, t % 3 - 1
                        for cnk in range(2):
                            b0 = 4 * cnk + 1 + dh
                            nc.tensor.matmul(
                                vp[:, cnk * 512:(cnk + 1) * 512],
                                lhsT=vdiag[:, qb, t, :],
                                rhs=v_b[:, qb, b0:b0 + 4, 1 + dw:1 + dw + W],
                                start=(pi == 0),
                                stop=(pi == len(V_TAPS_PE) - 1))

                y = ypool.tile([128, VB, S, W], BF16, tag="y", name=f"y{s}")
                for ti, t in enumerate(taps_dve):
                    dh, dw = t // 3 - 1, t % 3 - 1
                    if ti == 0:
                        for qb in range(VB):
                            nc.vector.tensor_scalar(
                                out=y[:, qb, :, :],
                                in0=v_b[:, qb, 1 + dh:1 + dh + S,
                                        1 + dw:1 + dw + W],
                                scalar1=w9_sb[:, (KB + qb) * 9 + t:
                                              (KB + qb) * 9 + t + 1],
                                scalar2=None, op0=ALU.mult)
                        continue
                    y2 = ypool.tile([128, VB, S, W], BF16, tag="y2",
                                    name=f"y2{s}_{ti}")
                    for qb in range(VB):
                        nc.vector.tensor_scalar(
                            out=y2[:, qb, :, :],
                            in0=v_b[:, qb, 1 + dh:1 + dh + S,
                                    1 + dw:1 + dw + W],
                            scalar1=w9_sb[:, (KB + qb) * 9 + t:
                                          (KB + qb) * 9 + t + 1],
                            scalar2=None, op0=ALU.mult)
                    nc.vector.tensor_tensor(
                        out=y[:], in0=y[:], in1=y2[:], op=ALU.add)
                # merge PE partial (PSUM) + DVE partial per block
                for qb in range(VB):
                    nc.vector.tensor_tensor(
                        out=v_sb[:, qb, r0:r0 + S, :],
                        in0=v_ps[qb][:].rearrange("p (r w) -> p r w", w=W),
                        in1=y[:, qb, :, :], op=ALU.add)

                # sumsq of q, k: ACT square+accum or DVE fused square+accum
                for ti, t_st in enumerate((q_st, k_st)):
                    for g in range(2):
                        if SUMSQ_ENG[ti * 2 + g] == "act":
                            nc.scalar.activation(
                                out=sq_scr[:, 0, :], in_=t_st[:, g, :, :],
                                func=ACTF.Square,
                                accum_out=stats[:, ti * 2 + g, s:s + 1])
                        else:
                            nc.vector.scalar_tensor_tensor(
                                out=sq_scr[:, 1, :].rearrange(
                                    "p (a b) -> p a b", b=W),
                                in0=t_st[:, g, :, :], scalar=1.0,
                                in1=t_st[:, g, :, :],
                                op0=ALU.mult, op1=ALU.mult,
                                accum_out=stats[:, ti * 2 + g, s:s + 1])

                # transpose q, k strips via the DMA crossbar (tile-major
                # [px, j, ch] layout; q/k share it so the attn contraction
                # enumeration stays consistent).
                qkT = tppool.tile([128, 2, 2, S, 128], BF16, tag="qkT",
                                  name=f"qkT{s}")
                for g in range(2):
                    nc.sync.dma_start_transpose(qkT[:, g], qk_st[:, g])
                tp_tiles[s] = (qkT[:, :, 0], qkT[:, :, 1])

            def emit_gram(s):
                # emitted one strip late so the PE queue never waits on the
                # transposes (head-of-line blocking).
                qT, kT = tp_tiles.pop(s)
                for g in range(2):
                    for j in range(S):
                        nc.tensor.matmul(
                            attn_ps[g][:],
                            lhsT=qT[:, g, j, :], rhs=kT[:, g, j, :],
                            start=(s == 0 and j == 0),
                            stop=(s == NSTRIP - 1 and j == S - 1))

            tp_tiles = {}
            spart = smalls[:, 56:60]
            emit_qkv(0)
            build_diag_tiles()
            for s in range(1, NSTRIP):
                emit_qkv(s)
                emit_rest(s - 1)
                if s >= 2:
                    emit_gram(s - 2)
            emit_rest(NSTRIP - 1)
            emit_gram(NSTRIP - 2)
            emit_gram(NSTRIP - 1)

            # ---- softmax + normalization scales ----
            ssq = smalls[:, 0:4]
            nrm = smalls[:, 4:8]
            for col in range(4):
                nc.vector.tensor_reduce(
                    out=spart[:, col:col + 1], in_=stats[:, col, 0:NSTRIP - 1],
                    axis=AX.X, op=ALU.add)
            nc.vector.tensor_tensor(out=ssq[:], in0=spart[:],
                                    in1=stats[:, :, NSTRIP - 1], op=ALU.add)
            nc.scalar.activation(out=nrm[:], in_=ssq[:], func=ACTF.Sqrt)
            nc.vector.tensor_scalar_max(nrm[:], nrm[:], EPS)
            rq = smalls[:, 8:10]
            nc.vector.reciprocal(out=rq[:], in_=nrm[:, 0:2])
            srow = smalls[:, 10:12]
            nc.vector.tensor_mul(srow[:], rq[:], temp_sb[:])

            k_nrm_t = smalls[:, 12:14]
            nc.vector.tensor_copy(out=k_nrm_t[:], in_=nrm[:, 2:4])
            tp_ps = ps_mm.tile([128, 512], FP32, tag="mmps",
                               name="tpps")[:, 0:128]
            nc.tensor.transpose(tp_ps[0:2, :], in_=k_nrm_t[:], identity=ident_f32[:])
            krow = persist.tile([128, 128], FP32, tag="krow")
            nc.vector.reciprocal(out=krow[0:2, :], in_=tp_ps[0:2, :])
            bc_k = persist.tile([128, 2, 32], FP32, tag="bck")
            for g in range(2):
                bc_ps = ps_mm.tile([128, 512], FP32, tag="mmps",
                                   name=f"bcps{g}")[:, 0:128]
                nc.tensor.matmul(bc_ps[:], lhsT=sel_row[:, g, :],
                                 rhs=krow[0:2, :], start=True, stop=True)
                for hh in range(4):
                    pr = slice(hh * 32, hh * 32 + 32)
                    nc.vector.tensor_copy(out=bc_k[pr, g, :],
                                          in_=bc_ps[pr, hh * 32:hh * 32 + 32])

            # pack each head's diagonal 32x32 block (scaled by 1/||q||*temp)
            # into [128, 2, 32], softmax all 4 heads per group at once.
            attn_sc = persist.tile([128, 2, 32], FP32, tag="attnsc")
            e_pk = persist.tile([128, 2, 32], FP32, tag="epk")
            for g in range(2):
                for hh in range(4):
                    pr = slice(hh * 32, hh * 32 + 32)
                    nc.vector.tensor_scalar(
                        out=attn_sc[pr, g, :],
                        in0=attn_ps[g][pr, hh * 32:hh * 32 + 32],
                        scalar1=srow[pr, g:g + 1], scalar2=None, op0=ALU.mult)
            for g in range(2):
                sm = attn_sc[:, g, :]
                nc.vector.tensor_mul(sm, sm, bc_k[:, g, :])
                mx = smalls[:, 48:49]
                nc.vector.tensor_reduce(out=mx, in_=sm, axis=AX.X, op=ALU.max)
                nmx = smalls[:, 49:50]
                nc.vector.tensor_scalar_mul(nmx, mx, -1.0)
                nc.scalar.activation(out=e_pk[:, g, :], in_=sm, func=ACTF.Exp,
                                     bias=nmx)
                ssum = smalls[:, 50:51]
                nc.vector.tensor_reduce(out=ssum, in_=e_pk[:, g, :],
                                        axis=AX.X, op=ALU.add)
                rsum = smalls[:, 51:52]
                nc.vector.reciprocal(out=rsum, in_=ssum)
                nc.vector.tensor_scalar(
                    out=e_pk[:, g, :], in0=e_pk[:, g, :],
                    scalar1=rsum, scalar2=None, op0=ALU.mult)
            for g in range(2):
                for hh in range(4):
                    pr = slice(hh * 32, hh * 32 + 32)
                    nc.vector.tensor_copy(
                        out=bd_pre[pr, g, hh * 32:hh * 32 + 32],
                        in_=e_pk[pr, g, :])

            for g in range(2):
                pat_ps = ps_mm.tile([128, 512], FP32, tag="mmps",
                                    name=f"patps{g}")[:, 0:256]
                nc.tensor.matmul(pat_ps[:], lhsT=bd_pre[:, g, :],
                                 rhs=wp_sb[:, g, :], start=True, stop=True)
                nc.vector.tensor_copy(out=pat[:, g, :], in_=pat_ps[:])

        # ---- phase 2: out = (P @ attn) @ v ----
        p2 = ExitStack()
        with p2:
            outpool = p2.enter_context(tc.tile_pool(name="outpool", bufs=8))
            ps_pj = p2.enter_context(tc.tile_pool(name="ps_pj", bufs=4, space="PSUM"))

            for cnk in range(H * W // 512):
                rr = cnk * 4
                pj_ps = ps_pj.tile([128, 2, 512], FP32, tag="pjps",
                                   name=f"pjps{cnk}")
                for ob in range(2):
                    for g in range(2):
                        nc.tensor.matmul(
                            pj_ps[:, ob, :],
                            lhsT=pat[:, g, ob * 128:(ob + 1) * 128],
                            rhs=v_sb[:, g, rr:rr + 4, :],
                            start=(g == 0), stop=(g == 1))
                o_sb = outpool.tile([128, 2, 512], BF16, tag="osb",
                                    name=f"osb{cnk}")
                if cnk % 2 == 0:
                    nc.vector.tensor_copy(out=o_sb[:], in_=pj_ps[:])
                else:
                    nc.scalar.copy(out=o_sb[:], in_=pj_ps[:])
                nc.sync.dma_start(
                    out=out_d[:].rearrange("(b p) h w -> p b h w", b=2)
                               [:, :, rr:rr + 4, :],
                    in_=o_sb[:].rearrange("p b (r w) -> p b r w", w=W))


_NC_CACHE = {}


def _get_nc():
    if "nc" not in _NC_CACHE:
        _NC_CACHE["nc"] = build_kernel()
    return _NC_CACHE["nc"]


def _host_prep(w_qkv, w_dw, w_proj, temperature):
    w_qkvT_f32 = np.ascontiguousarray(np.asarray(w_qkv, dtype=np.float32).T)
    w_qkvT = w_qkvT_f32.astype(ml_dtypes.bfloat16)
    # fp8 DoubleRow qkv weights for q,k blocks: [p, kb, qb*128+c]
    wq8 = np.ascontiguousarray(
        (w_qkvT_f32[:, 0:KB * 128] * W8_SCALE)
        .reshape(2, 128, KB * 128).transpose(1, 0, 2)
    ).astype(ml_dtypes.float8_e4m3)
    w9f = np.asarray(w_dw, dtype=np.float32)[:, 0, 1]          # [768, 3, 3]
    w9 = np.empty((128, QB * 9), dtype=np.float32)
    for b in range(QB):
        w9[:, b * 9:(b + 1) * 9] = w9f[b * 128:(b + 1) * 128].reshape(128, 9)
    # center-tap lo weights: 16*w4 - fp8(16*w4), per q,k block
    w4lo = np.empty((128, KB), dtype=np.float32)
    for b in range(KB):
        w16 = w9[:, b * 9 + 4] * W_SCALE
        hi = np.asarray(w16.astype(ml_dtypes.float8_e4m3), np.float32)
        w4lo[:, b] = w16 - hi
    w_projT = np.ascontiguousarray(np.asarray(w_proj, dtype=np.float32).T)
    w_projT_bf = w_projT.astype(ml_dtypes.bfloat16)
    sel2 = np.zeros((2, 2, 128), dtype=np.float32)
    sel2[0, 0, :] = 1.0
    sel2[1, 1, :] = 1.0
    t = np.asarray(temperature, dtype=np.float32).reshape(HEADS)
    temp_pc = np.empty((128, 2), dtype=np.float32)
    for g in range(2):
        for p in range(128):
            temp_pc[p, g] = t[(g * 128 + p) // CH]
    return w_qkvT, wq8, w9, w4lo, w_projT_bf, temp_pc, sel2


def build_in_maps(x, w_qkv, w_dw, w_proj, temperature):
    x = np.asarray(x, dtype=np.float32)
    b = x.shape[0]
    assert b == N_CORES
    w_qkvT, wq8, w9, w4lo, w_projT_bf, temp_pc, sel2 = _host_prep(
        w_qkv, w_dw, w_proj, temperature)
    x_bf = np.ascontiguousarray(x).astype(ml_dtypes.bfloat16)
    x8 = np.ascontiguousarray(x.reshape(b, 2, 128, H, W)).astype(
        ml_dtypes.float8_e4m3)
    return [{
        "x": x_bf[i],
        "x8": x8[i],
        "wq8": wq8,
        "w_qkvT": w_qkvT,
        "w9": w9,
        "w4lo": w4lo,
        "w_projT": w_projT_bf,
        "temp_pc": temp_pc,
        "sel2": sel2,
    } for i in range(b)]


def kernel(x, w_qkv, w_dw, w_proj, temperature):
    from concourse.bass_utils import run_bass_kernel_spmd

    b = np.asarray(x).shape[0]
    in_maps = build_in_maps(x, w_qkv, w_dw, w_proj, temperature)
    nc = _get_nc()
    res = run_bass_kernel_spmd(nc, in_maps, core_ids=list(range(N_CORES)))
    out = np.stack([np.asarray(res.results[i]["out"], dtype=np.float32)
                    for i in range(b)], axis=0)
    return out


if __name__ == "__main__":
    nc = build_kernel()
    print("built + compiled OK")

